# revision 1
# baseline (speedup 1.0000x reference)
"""CentralDiff2D (submanifold 3x3 conv, central difference along x) on 8 trn2
NeuronCores.

Sharding strategy (grid-partitioned / sort-based spatial tiling):
  The stencil touches cells (x-1,y) and (x+1,y) only, so the neighbor of a
  point is active iff the point at grid-linear index lin +- 1 (lin = y*W + x)
  is occupied.  The host shards by sorting points in grid-linear order and
  splitting into 8 equal shards (equivalent to partitioning the grid by rows
  into 8 balanced bands, with a 1-point halo at each shard boundary).

  Each core receives its shard as [128, F+2] arrays (lin, f) where each SBUF
  partition row carries its own 2-element halo, so on device the left/right
  sorted neighbors of every point are free-dim offset slices.  The device
  computes, fully dense:

      active_r = (lin[i+1] == lin[i] + 1) & (x != W-1)
      active_l = (lin[i] == lin[i-1] + 1) & (x != 0)
      out[i]   = 0.5 * (f[i+1] * active_r - f[i-1] * active_l)

  which is exactly the reference semantics for unique active sites.  The host
  then inverse-permutes the concatenated shard outputs back to input order.
"""
import numpy as np

import bass_rust
import concourse.bass as bass
import concourse.mybir as mybir
import concourse.tile as tile
from concourse.bass_utils import run_bass_kernel_spmd

P = 128
NCORES = 8
W_GRID = 4096
N_POINTS = 4_000_000
C_SHARD = N_POINTS // NCORES          # 500000 points per core
F = 3908                              # ceil(C_SHARD / P), free dim per partition
NPC = P * F                           # padded shard capacity (500224)
SENT_HI = 1 << 26
SENT_LO = -(1 << 26)

_MAX_WAITS = 1  # this toolchain's walrus rejects >1 sync wait per instruction


def _split_multiwaits(nc, max_waits=_MAX_WAITS):
    ctr = 0
    for f in nc.m.functions:
        for bb in f.blocks:
            insts = bb.instructions
            out = []
            for inst in insts:
                si = inst.sync_info
                if si is not None and si.on_wait and len(si.on_wait) > max_waits:
                    waits = list(si.on_wait)
                    head, tail = waits[:-max_waits], waits[-max_waits:]
                    for j in range(0, len(head), max_waits):
                        nop = mybir.InstNoOp(name=f"I-msplit-{ctr}", ins=[], outs=[])
                        ctr += 1
                        nop.engine = inst.engine
                        nop.sync_info = mybir.SyncInfo(
                            on_wait=head[j:j + max_waits], on_update=[])
                        out.append(nop)
                    si.on_wait = tail
                out.append(inst)
            if len(out) != len(insts):
                bb.instructions[:] = out
                assert len(bb.instructions) == len(out), \
                    "bb.instructions slice-assign did not persist"


def build_kernel(reps=1):
    """Per-core device kernel: sorted-adjacency central difference."""
    nc = bass.Bass()
    ll_in = nc.dram_tensor("ll", [P, F + 2], mybir.dt.int32, kind="ExternalInput")
    f_in = nc.dram_tensor("f", [P, F + 2], mybir.dt.float32, kind="ExternalInput")
    vals_out = nc.dram_tensor("vals", [P, F], mybir.dt.float32,
                              kind="ExternalOutput")
    AT = mybir.AluOpType

    with tile.TileContext(nc) as tc:
        with tc.tile_pool(name="io", bufs=1) as iop, \
             tc.tile_pool(name="work", bufs=1) as wp:
            for _r in range(reps):
                L = iop.tile([P, F + 2], mybir.dt.int32, tag="L")
                Ff = iop.tile([P, F + 2], mybir.dt.float32, tag="Ff")
                nc.sync.dma_start(out=L[:], in_=ll_in[:])
                nc.sync.dma_start(out=Ff[:], in_=f_in[:])

                Lp, Lc, Ln = L[:, 0:F], L[:, 1:F + 1], L[:, 2:F + 2]
                Fp, Fn = Ff[:, 0:F], Ff[:, 2:F + 2]

                d1 = wp.tile([P, F], mybir.dt.int32, tag="d1")
                d0 = wp.tile([P, F], mybir.dt.int32, tag="d0")
                xb = wp.tile([P, F], mybir.dt.int32, tag="xb")
                m1 = wp.tile([P, F], mybir.dt.float32, tag="m1")
                m0 = wp.tile([P, F], mybir.dt.float32, tag="m0")
                mr = wp.tile([P, F], mybir.dt.float32, tag="mr")
                mlk = wp.tile([P, F], mybir.dt.float32, tag="mlk")
                t1 = wp.tile([P, F], mybir.dt.float32, tag="t1")
                t2 = wp.tile([P, F], mybir.dt.float32, tag="t2")
                vt = wp.tile([P, F], mybir.dt.float32, tag="vt")

                nc.vector.tensor_tensor(out=d1[:], in0=Ln, in1=Lc, op=AT.subtract)
                nc.vector.tensor_tensor(out=d0[:], in0=Lc, in1=Lp, op=AT.subtract)
                nc.vector.tensor_scalar(out=xb[:], in0=Lc, scalar1=W_GRID - 1,
                                        scalar2=None, op0=AT.bitwise_and)
                nc.vector.tensor_scalar(out=m1[:], in0=d1[:], scalar1=1,
                                        scalar2=None, op0=AT.is_equal)
                nc.vector.tensor_scalar(out=m0[:], in0=d0[:], scalar1=1,
                                        scalar2=None, op0=AT.is_equal)
                nc.vector.tensor_scalar(out=mr[:], in0=xb[:], scalar1=W_GRID - 1,
                                        scalar2=None, op0=AT.not_equal)
                nc.vector.tensor_scalar(out=mlk[:], in0=xb[:], scalar1=0,
                                        scalar2=None, op0=AT.not_equal)
                nc.vector.tensor_tensor(out=m1[:], in0=m1[:], in1=mr[:], op=AT.mult)
                nc.vector.tensor_tensor(out=m0[:], in0=m0[:], in1=mlk[:], op=AT.mult)
                nc.vector.tensor_tensor(out=t1[:], in0=Fn, in1=m1[:], op=AT.mult)
                nc.vector.tensor_tensor(out=t2[:], in0=Fp, in1=m0[:], op=AT.mult)
                nc.vector.tensor_tensor(out=t1[:], in0=t1[:], in1=t2[:],
                                        op=AT.subtract)
                nc.vector.tensor_scalar(out=vt[:], in0=t1[:], scalar1=0.5,
                                        scalar2=None, op0=AT.mult)
                nc.sync.dma_start(out=vals_out[:], in_=vt[:])

    _split_multiwaits(nc)
    return nc


_NC_CACHE = {}


def _get_nc(reps=1):
    if reps not in _NC_CACHE:
        _NC_CACHE[reps] = build_kernel(reps)
    return _NC_CACHE[reps]


def _shard_inputs(lin_sorted, f_sorted):
    """Build per-core [128, F+2] halo-strided arrays."""
    in_maps = []
    for k in range(NCORES):
        lo, hi = k * C_SHARD, (k + 1) * C_SHARD
        Bl = np.full(NPC + 2, SENT_HI, np.int32)
        Bf = np.zeros(NPC + 2, np.float32)
        Bl[1:C_SHARD + 1] = lin_sorted[lo:hi]
        Bf[1:C_SHARD + 1] = f_sorted[lo:hi]
        if k > 0:
            Bl[0] = lin_sorted[lo - 1]
            Bf[0] = f_sorted[lo - 1]
        else:
            Bl[0] = SENT_LO
        if k < NCORES - 1:
            Bl[C_SHARD + 1] = lin_sorted[hi]
            Bf[C_SHARD + 1] = f_sorted[hi]
        l2d = np.lib.stride_tricks.as_strided(
            Bl, (P, F + 2), (F * 4, 4)).copy()
        f2d = np.lib.stride_tricks.as_strided(
            Bf, (P, F + 2), (F * 4, 4)).copy()
        in_maps.append({"ll": l2d, "f": f2d})
    return in_maps


def kernel(coords, feats, H, W):
    H, W = int(H), int(W)
    assert H == 4096 and W == 4096, (H, W)
    coords = np.asarray(coords)
    feats = np.asarray(feats)
    n = coords.shape[0]
    assert n == N_POINTS, n

    x = coords[:, 0].astype(np.int64)
    y = coords[:, 1].astype(np.int64)
    lin = (y * W + x).astype(np.int32)

    order = np.argsort(lin, kind="stable")
    lin_sorted = lin[order]
    f_sorted = np.ascontiguousarray(feats[:, 0].astype(np.float32)[order])

    in_maps = _shard_inputs(lin_sorted, f_sorted)
    nc = _get_nc(reps=1)
    res = run_bass_kernel_spmd(nc, in_maps, core_ids=list(range(NCORES)))

    out_sorted = np.empty(n, np.float32)
    for k in range(NCORES):
        out_sorted[k * C_SHARD:(k + 1) * C_SHARD] = \
            res.results[k]["vals"].ravel()[:C_SHARD]
    out = np.empty(n, np.float32)
    out[order] = out_sorted
    return out[:, None]


# revision 6
# speedup vs baseline: 142.6882x; 142.6882x over previous
"""CentralDiff2D (submanifold 3x3 conv, central difference along x) on 8 trn2
NeuronCores.

Sharding strategy (grid-partitioned / sort-based spatial tiling):
  The stencil touches cells (x-1,y) and (x+1,y) only, so the neighbor of a
  point is active iff the point at grid-linear index lin +- 1 (lin = y*W + x)
  is occupied.  The host shards by sorting points in grid-linear order and
  splitting into 8 equal shards (equivalent to partitioning the grid by rows
  into 8 balanced bands, with a 1-point halo at each shard boundary).

  Points are relabelled with the row-weighted key V = lin + (lin & ~(W-1)).
  For sorted unique lins, V[i+1] - V[i] == 1 iff the next point is the
  (x+1, y) grid neighbor (the doubled row term makes any row crossing push
  the difference past 1, which also covers the x == W-1 / x == 0 boundary
  masks of the reference).

  Each core receives its shard as [128, F+2] arrays (V, f) where each SBUF
  partition row carries its own 2-element halo, so the left/right sorted
  neighbors of every point are free-dim offset slices.  The device computes,
  fully dense and pipelined in chunks:

      dd[i] = V[i+1] - V[i]
      out[i] = (0.5 * (dd[i+1] == 1)) * f[i+1] - (0.5 * (dd[i] == 1)) * f[i-1]

  which is exactly the reference semantics for unique active sites.  The host
  then inverse-permutes the concatenated shard outputs back to input order.
"""
import numpy as np

import concourse.bass as bass
import concourse.mybir as mybir
import concourse.tile as tile
from concourse.bass_utils import run_bass_kernel_spmd

P = 128
NCORES = 8
W_GRID = 4096
N_POINTS = 4_000_000
C_SHARD = N_POINTS // NCORES          # 500000 points per core
F = 3968                              # free dim per partition (31 * 128)
NPC = P * F                           # padded shard capacity (507904)
NCHUNK = 2
CH = F // NCHUNK                      # 1984 output columns per chunk
SENT_HI = 1 << 26
SENT_LO = -(1 << 26)

_MAX_WAITS = 1  # this toolchain's walrus rejects >1 sync wait per instruction


def _split_multiwaits(nc, max_waits=_MAX_WAITS):
    ctr = 0
    for fn in nc.m.functions:
        for bb in fn.blocks:
            insts = bb.instructions
            out = []
            for inst in insts:
                si = inst.sync_info
                if si is not None and si.on_wait and len(si.on_wait) > max_waits:
                    waits = list(si.on_wait)
                    head, tail = waits[:-max_waits], waits[-max_waits:]
                    for j in range(0, len(head), max_waits):
                        nop = mybir.InstNoOp(name=f"I-msplit-{ctr}", ins=[], outs=[])
                        ctr += 1
                        nop.engine = inst.engine
                        nop.sync_info = mybir.SyncInfo(
                            on_wait=head[j:j + max_waits], on_update=[])
                        out.append(nop)
                    si.on_wait = tail
                out.append(inst)
            if len(out) != len(insts):
                bb.instructions[:] = out
                assert len(bb.instructions) == len(out), \
                    "bb.instructions slice-assign did not persist"


def build_kernel(reps=1, use_loop=False):
    """Per-core device kernel: sorted-adjacency central difference.

    use_loop=True wraps the body in a hardware For_i loop of `reps`
    iterations (used only for repeat-delta timing in test.py).
    """
    import contextlib

    nc = bass.Bass()
    v_in = nc.dram_tensor("v", [P, F + 2], mybir.dt.int32, kind="ExternalInput")
    f_in = nc.dram_tensor("f", [P, F + 2], mybir.dt.float32, kind="ExternalInput")
    vals_out = nc.dram_tensor("vals", [P, F], mybir.dt.float32,
                              kind="ExternalOutput")
    AT = mybir.AluOpType

    with tile.TileContext(nc) as tc:
        with tc.tile_pool(name="work", bufs=3) as wp:
            loop_cm = tc.For_i(0, reps) if use_loop else contextlib.nullcontext()
            with loop_cm:
                body_reps = 1 if use_loop else reps
                _emit_body(nc, tc, wp, v_in, f_in, vals_out, AT, body_reps)

    _split_multiwaits(nc)
    return nc


def _emit_body(nc, tc, wp, v_in, f_in, vals_out, AT, reps):
    for _r in range(reps):
        for c in range(NCHUNK):
            c0 = c * CH
            Lv = wp.tile([P, CH + 2], mybir.dt.int32, tag="Lv")
            Fv = wp.tile([P, CH + 2], mybir.dt.float32, tag="Fv")
            nc.sync.dma_start(out=Lv[:], in_=v_in[:, c0:c0 + CH + 2])
            nc.sync.dma_start(out=Fv[:], in_=f_in[:, c0:c0 + CH + 2])

            dd = wp.tile([P, CH + 1], mybir.dt.int32, tag="dd")
            m1 = wp.tile([P, CH], mybir.dt.float32, tag="m1")
            m0 = wp.tile([P, CH], mybir.dt.float32, tag="m0")
            vo = wp.tile([P, CH], mybir.dt.float32, tag="vo")
            nc.vector.tensor_tensor(
                out=dd[:], in0=Lv[:, 1:CH + 2], in1=Lv[:, 0:CH + 1],
                op=AT.subtract)
            nc.vector.tensor_scalar(
                out=m1[:], in0=dd[:, 1:CH + 1], scalar1=1, scalar2=0.5,
                op0=AT.is_equal, op1=AT.mult)
            nc.vector.tensor_scalar(
                out=m0[:], in0=dd[:, 0:CH], scalar1=1, scalar2=0.5,
                op0=AT.is_equal, op1=AT.mult)
            nc.vector.tensor_tensor(
                out=m1[:], in0=Fv[:, 2:CH + 2], in1=m1[:], op=AT.mult)
            nc.vector.tensor_tensor(
                out=m0[:], in0=Fv[:, 0:CH], in1=m0[:], op=AT.mult)
            nc.vector.tensor_tensor(
                out=vo[:], in0=m1[:], in1=m0[:], op=AT.subtract)
            nc.sync.dma_start(out=vals_out[:, c0:c0 + CH], in_=vo[:])


_NC_CACHE = {}


def _get_nc(reps=1):
    if reps not in _NC_CACHE:
        _NC_CACHE[reps] = build_kernel(reps)
    return _NC_CACHE[reps]


def _shard_inputs(v_sorted, f_sorted):
    """Build per-core [128, F+2] halo-strided arrays."""
    in_maps = []
    for k in range(NCORES):
        lo, hi = k * C_SHARD, (k + 1) * C_SHARD
        # Rebase V per shard: the DVE evaluates int32 ALU ops via fp32, which
        # is exact only below 2^24.  Shard-local offsets stay < 2^23.
        base = np.int32(v_sorted[lo])
        Bv = np.full(NPC + 2, SENT_HI, np.int32)
        Bf = np.zeros(NPC + 2, np.float32)
        Bv[1:C_SHARD + 1] = v_sorted[lo:hi] - base
        Bf[1:C_SHARD + 1] = f_sorted[lo:hi]
        if k > 0:
            Bv[0] = v_sorted[lo - 1] - base
            Bf[0] = f_sorted[lo - 1]
        else:
            Bv[0] = SENT_LO
        if k < NCORES - 1:
            Bv[C_SHARD + 1] = v_sorted[hi] - base
            Bf[C_SHARD + 1] = f_sorted[hi]
        v2d = np.lib.stride_tricks.as_strided(
            Bv, (P, F + 2), (F * 4, 4)).copy()
        f2d = np.lib.stride_tricks.as_strided(
            Bf, (P, F + 2), (F * 4, 4)).copy()
        in_maps.append({"v": v2d, "f": f2d})
    return in_maps


def kernel(coords, feats, H, W):
    H, W = int(H), int(W)
    assert H == 4096 and W == 4096, (H, W)
    coords = np.asarray(coords)
    feats = np.asarray(feats)
    n = coords.shape[0]
    assert n == N_POINTS, n

    x = coords[:, 0].astype(np.int64)
    y = coords[:, 1].astype(np.int64)
    lin = (y * W + x).astype(np.int32)

    order = np.argsort(lin, kind="stable")
    lin_sorted = lin[order]
    v_sorted = lin_sorted + (lin_sorted & ~np.int32(W - 1))
    f_sorted = np.ascontiguousarray(feats[:, 0].astype(np.float32)[order])

    in_maps = _shard_inputs(v_sorted, f_sorted)
    nc = _get_nc(reps=1)
    res = run_bass_kernel_spmd(nc, in_maps, core_ids=list(range(NCORES)))

    out_sorted = np.empty(n, np.float32)
    for k in range(NCORES):
        out_sorted[k * C_SHARD:(k + 1) * C_SHARD] = \
            res.results[k]["vals"].ravel()[:C_SHARD]
    out = np.empty(n, np.float32)
    out[order] = out_sorted
    return out[:, None]
